# revision 2
# baseline (speedup 1.0000x reference)
"""GCN on 8 trn2 NeuronCores via Bass/Tile.

Layout/algorithm (validated in proto.py):
- Nodes sharded by id range across 8 cores; within a core, nodes are
  degree-sorted (perm pi) and padded to nb*128.
- Table trick: each conv layer builds tableS[v] = dinv[v] * (h[v] @ W) as a
  per-core shard, AllGathers the full bf16 table to DRAM, then each core
  gathers rows for its (dst-sorted, degree-bucketed) edge slots with
  dma_gather and segment-reduces on the vector engine.
- agg[d] = dinv[d] * (sum_slots table[src]); conv bias is a no-op under
  training-mode BN; BN scale folds into the next weight matrix, BN shift
  folds into a per-feature delta added before ReLU.
- BN stats (sum, sum of squares) via ones-matmul on PE + tiny AllReduce.
- Table DRAM layout: per core `shard = (nb+1)*128` rows; node at sorted pos
  i lives at flat row c*shard + (i%128)*(nb+1) + (i//128); rows with b==nb
  are zero rows (padding targets). dma_gather needs a 256-byte HBM row
  stride and int16 (<32768) indices, so flat rows are grouped into packed
  256B rows of row_div = 128//F nodes; indices address packed rows within a
  <=25344-row stripe, and the sub-row is selected by the gather source AP's
  element offset.  Slots are therefore split into
  n_classes = n_stripes*row_div classes; within each (class, group of
  consecutive dst blocks) the per-dst slot count is padded to a uniform
  width so one strided reduce handles the whole group.
- dma_gather quirks (found empirically): idx position i lives at
  [i%16, i//16] of a [128, n/16] int16 SBUF tile and must be replicated
  across all eight 16-partition stripes; <=1024 indices per call; payload
  may be a 64/16-element slice of the 128-element row (elem_step=128).
"""
import numpy as np
import ml_dtypes

import concourse.bass as bass
import concourse.bacc as bacc
import concourse.mybir as mybir
import concourse.tile as tile
from concourse.bass_utils import run_bass_kernel_spmd

BF16 = ml_dtypes.bfloat16
P = 128
NCORES = 8
BN_EPS = 1e-5
KMAX = 96            # max gather-tile columns per (class, group)
CALL_COLS = 8        # columns (=128 idxs each) per dma_gather call
IDX_CAP = 32768      # int16 index space per stripe


# ----------------------------------------------------------------- host prep

def _plan_classes(F, nb, shard):
    row_div = 128 // F
    packed_rows = NCORES * shard // row_div
    n_stripes = -(-packed_rows // IDX_CAP)
    assert packed_rows % n_stripes == 0
    stripe = packed_rows // n_stripes
    return dict(F=F, row_div=row_div, n_stripes=n_stripes, stripe=stripe,
                n_classes=n_stripes * row_div)


def _pad_lidx(pl, nb, shard):
    """Per class: list of local packed-row idxs that are zero half-rows."""
    row_div, stripe = pl["row_div"], pl["stripe"]
    cps = NCORES // pl["n_stripes"]          # cores per stripe
    pads = []
    for k in range(pl["n_classes"]):
        s, pi = divmod(k, row_div)
        c0 = s * cps
        lids = []
        for p in range(P):
            r = c0 * shard + p * (nb + 1) + nb
            if r % row_div == pi:
                lids.append(r // row_div - s * stripe)
        assert lids, (k, pi)
        pads.append(np.array(lids, np.int64))
    return pads


def prep(x, edge_index):
    N = x.shape[0]
    assert N % NCORES == 0
    n_own = N // NCORES
    nb = (n_own + P - 1) // P
    npad = nb * P
    shard = (nb + 1) * P

    src = np.asarray(edge_index[0], np.int64)
    dst = np.asarray(edge_index[1], np.int64)
    deg = np.bincount(dst, minlength=N).astype(np.int64) + 1
    dinv = (1.0 / np.sqrt(deg.astype(np.float64))).astype(np.float32)

    perms = []
    inv_pos = np.empty(N, np.int64)
    for c in range(NCORES):
        lo = c * n_own
        order = np.argsort(-deg[lo:lo + n_own], kind="stable")
        perms.append(order)
        inv = np.empty(n_own, np.int64)
        inv[order] = np.arange(n_own)
        inv_pos[lo:lo + n_own] = inv

    # flat table row of each global node
    gi = np.arange(N)
    rflat = ((gi // n_own) * shard + (inv_pos % P) * (nb + 1)
             + inv_pos // P).astype(np.int64)

    # edge arrays sorted by (dst core, dst pos)
    ecore = dst // n_own
    eorder = np.argsort(inv_pos[dst] + (ecore << 32), kind="stable")
    ds = dst[eorder]
    ss = src[eorder]
    e_ci = ds // n_own                      # dst core
    e_i = inv_pos[ds]                       # dst sorted pos
    e_r = rflat[ss]                         # src flat row

    plans = {}
    idx_arrays = {}
    for F in (64, 16):
        pl = _plan_classes(F, nb, shard)
        rd, stripe, ncl = pl["row_div"], pl["stripe"], pl["n_classes"]
        pads = _pad_lidx(pl, nb, shard)

        e_pk = e_r // rd
        e_s = e_pk // stripe
        e_k = e_s * rd + e_r % rd           # class of each edge's src
        e_l = e_pk - e_s * stripe           # local idx within stripe

        self_r = np.empty((NCORES, npad), np.int64)
        for c in range(NCORES):
            i = np.arange(npad)
            self_r[c] = c * shard + (i % P) * (nb + 1) + i // P
        self_pk = self_r // rd
        self_s = self_pk // stripe
        self_k = self_s * rd + self_r % rd
        self_l = self_pk - self_s * stripe

        # per-(core, node, class) counts
        cnt = np.zeros((NCORES, npad, ncl), np.int32)
        np.add.at(cnt, (e_ci, e_i, e_k), 1)
        real = np.arange(npad) < n_own
        cnt[:, real, :][...] = cnt[:, real, :]          # no-op, clarity
        for c in range(NCORES):
            cnt[c, real, self_k[c, real]] += 1

        # uniform widths per (class, block) over all cores
        Wbk = cnt.reshape(NCORES, nb, P, ncl).max(axis=(0, 2))   # [nb, ncl]

        # group packing per class: uniform width per group, <= KMAX cols
        groups = []          # (k, b0, nblk, Wg, col0)
        col0 = 0
        for k in range(ncl):
            b = 0
            while b < nb:
                b0 = b
                wmax = max(int(Wbk[b, k]), 1)
                n = 1
                b += 1
                while b < nb and n < 24:
                    w2 = max(wmax, int(Wbk[b, k]))
                    if (n + 1) * w2 > KMAX:
                        break
                    wmax = w2
                    n += 1
                    b += 1
                groups.append((k, b0, n, wmax, col0))
                col0 += n * wmax
        T = col0

        # column base for each (core-independent) (class, block)
        colbase = np.full((ncl, nb), -1, np.int64)
        gwidth = np.zeros((ncl, nb), np.int64)
        for (k, b0, n, wg, c0) in groups:
            for j in range(n):
                colbase[k, b0 + j] = c0 + j * wg
                gwidth[k, b0 + j] = wg

        # fill idx columns per core
        idxs = np.empty((NCORES, P, T), np.int16)
        padcols = np.empty(T, np.int16)
        for (kk, b0, n, wg, c0) in groups:
            padv = pads[kk]
            w = np.arange(n * wg)
            padcols[c0:c0 + n * wg] = padv[w % len(padv)].astype(np.int16)
        for c in range(NCORES):
            idxs[c][:] = padcols[None, :]
            # self slots
            i = np.arange(n_own)
            b = i // P
            p = i % P
            k = self_k[c, :n_own]
            idxs[c][p, colbase[k, b]] = self_l[c, :n_own].astype(np.int16)
        # edge slots (vectorized across all cores)
        key2 = (e_ci << 56) + (e_i << 8) + e_k
        o2 = np.argsort(key2, kind="stable")
        k2 = e_k[o2]
        ci2 = e_ci[o2]
        i2 = e_i[o2]
        l2 = e_l[o2]
        kk2 = key2[o2]
        newrun = np.concatenate([[True], kk2[1:] != kk2[:-1]])
        starts = np.flatnonzero(newrun)
        run_id = np.cumsum(newrun) - 1
        rank = np.arange(len(o2)) - starts[run_id]
        is_self_cls = self_k[ci2, i2] == k2
        col = colbase[k2, i2 // P] + rank + is_self_cls
        idxs[ci2, i2 % P, col] = l2.astype(np.int16)

        # pack for dma_gather: position p of column j -> [p%16, 8j + p//16],
        # replicated across the 8 partition stripes
        packed = np.empty((NCORES, P, 8 * T), np.int16)
        for c in range(NCORES):
            a = idxs[c].reshape(P, T)                       # [128, T]
            b = a.reshape(8, 16, T).transpose(1, 0, 2)      # [16, 8, T] (a=p%16,t=p//16)
            b = b.transpose(0, 2, 1).reshape(16, T * 8)     # [16, 8T] col-major per j
            packed[c] = np.tile(b, (8, 1))

        pl["groups"] = groups
        pl["T"] = T
        plans[F] = pl
        idx_arrays[F] = packed

    dinv_sb = np.zeros((NCORES, P, nb), np.float32)
    for c in range(NCORES):
        d = dinv[c * n_own:(c + 1) * n_own][perms[c]]
        dpad = np.concatenate([d, np.zeros(npad - n_own, np.float32)])
        dinv_sb[c] = dpad.reshape(nb, P).T

    meta = dict(N=N, n_own=n_own, nb=nb, npad=npad, shard=shard, plans=plans)
    return meta, perms, idx_arrays, dinv_sb


def make_core_inputs(meta, perms, idx_arrays, dinv_sb, inputs):
    x = np.asarray(inputs["x"], np.float32)
    n_own, nb, npad = meta["n_own"], meta["nb"], meta["npad"]
    DIN = x.shape[1]

    shared = {
        "w1": np.asarray(inputs["fc1_w"], np.float32).astype(BF16),
        "b1": np.tile(np.asarray(inputs["fc1_b"], np.float32), (P, 1)),
        "w3": np.asarray(inputs["fc2_w"], np.float32).astype(BF16),
        "b2": np.tile(np.asarray(inputs["fc2_b"], np.float32), (P, 1)),
        "idbf": np.eye(P, dtype=np.float32).astype(BF16),
        "ones_col": np.ones((P, 1), np.float32),
        "ones_row": np.ones((1, P), np.float32),
    }
    for l in range(3):
        shared[f"wc{l}"] = np.asarray(inputs[f"conv{l}_w"], np.float32).astype(BF16)
        shared[f"bng{l}"] = np.asarray(inputs[f"bn{l}_g"], np.float32)[None, :]
        shared[f"bnb{l}"] = np.asarray(inputs[f"bn{l}_b"], np.float32)[None, :]

    in_maps = []
    for c in range(NCORES):
        xs = x[c * n_own:(c + 1) * n_own][perms[c]]
        xs = np.concatenate([xs, np.zeros((npad - n_own, DIN), np.float32)])
        m = dict(shared)
        m["x_t"] = np.ascontiguousarray(xs.T).astype(BF16)
        m["idx64"] = np.ascontiguousarray(idx_arrays[64][c])
        m["idx16"] = np.ascontiguousarray(idx_arrays[16][c])
        m["dinv"] = np.ascontiguousarray(dinv_sb[c])
        in_maps.append(m)
    return in_maps


def unpack_outputs(meta, perms, results):
    N, n_own, nb = meta["N"], meta["n_own"], meta["nb"]
    C = 16
    out = np.empty((N, C), np.float32)
    for c in range(NCORES):
        o = np.asarray(results[c]["out"])
        o = o.reshape(P, nb, C).transpose(1, 0, 2).reshape(nb * P, C)[:n_own]
        tmp = np.empty((n_own, C), np.float32)
        tmp[perms[c]] = o
        out[c * n_own:(c + 1) * n_own] = tmp
    return out


# ------------------------------------------------------------ device program

def _stat_chunks(nb, F):
    kmax = 512 // F
    k = max(d for d in range(1, kmax + 1) if nb % d == 0)
    return nb // k, k * F


def _dma_gather_raw(gp, out_ap, in_ap, idxs_ap, num_idxs, elem_size,
                    elem_step, queue_num):
    """bass.dma_gather minus the elem_size%256 assert: payload may be a
    slice of the 256B-strided row."""
    stride_bytes = elem_step * mybir.dt.size(in_ap.dtype)
    assert stride_bytes % 256 == 0 and stride_bytes // 256 < 256
    assert idxs_ap.dtype == mybir.dt.int16
    return gp.add_instruction(
        mybir.InstDMAGatherAnt(
            name=gp.bass.get_next_instruction_name(),
            ins=[*gp.lower_ap_dma(in_ap, for_custom_bir_dma=True),
                 gp.lower_ap(idxs_ap),
                 gp.lower_val_access(gp.to_reg(num_idxs))],
            outs=[gp.lower_ap(out_ap)],
            transpose=False, num_idxs=num_idxs, elem_size=elem_size,
            stride_bytes_256=stride_bytes // 256, gen_mode=0,
            single_packet=True, queue_num=queue_num,
            sbuf_tokens_per_rank=0, sbuf_free_dim_per_rank=0,
            sbuf_free_dim_pad_per_rank=0, sbuf_byte_offset=0))


def build_program(meta, DIN=128, H=64, C=16):
    nb, npad, shard = meta["nb"], meta["npad"], meta["shard"]
    N = meta["N"]
    plans = meta["plans"]
    NSH = NCORES * shard
    dt = mybir.dt
    f32, bf16, i16 = dt.float32, dt.bfloat16, dt.int16
    RG = [list(range(NCORES))]

    nc = bacc.Bacc("TRN2", target_bir_lowering=False, debug=False,
                   num_devices=NCORES, num_swdge_queues=4)

    def din(name, shape, dtype):
        return nc.dram_tensor(name, shape, dtype, kind="ExternalInput").ap()

    xt_d = din("x_t", [DIN, npad], bf16)
    idx_d = {F: din(f"idx{F}", [P, 8 * plans[F]["T"]], i16) for F in (64, 16)}
    dinv_d = din("dinv", [P, nb], f32)
    w1_d = din("w1", [DIN, H], bf16)
    b1_d = din("b1", [P, H], f32)
    wc_d = [din(f"wc{l}", [H, H if l < 2 else C], bf16) for l in range(3)]
    bng_d = [din(f"bng{l}", [1, H if l < 2 else C], f32) for l in range(3)]
    bnb_d = [din(f"bnb{l}", [1, H if l < 2 else C], f32) for l in range(3)]
    w3_d = din("w3", [C, C], bf16)
    b2_d = din("b2", [P, C], f32)
    idbf_d = din("idbf", [P, P], bf16)
    onec_d = din("ones_col", [P, 1], f32)
    oner_d = din("ones_row", [1, P], f32)
    out_d = nc.dram_tensor("out", [P, nb * C], f32, kind="ExternalOutput").ap()

    with tile.TileContext(nc) as tc:
        with tc.tile_pool(name="pers", bufs=1) as pers, \
             tc.tile_pool(name="work", bufs=2) as work, \
             tc.tile_pool(name="gwork", bufs=3) as gwork, \
             tc.tile_pool(name="ps2", bufs=2, space="PSUM") as ps2, \
             tc.tile_pool(name="ps1", bufs=1, space="PSUM") as ps1, \
             tc.tile_pool(name="dram", bufs=1, space="DRAM") as dram:

            xt = pers.tile([DIN, npad], bf16, tag="xt", name="xt")
            dinv = pers.tile([P, nb], f32, tag="dinv", name="dinv")
            w1 = pers.tile([DIN, H], bf16, tag="w1", name="w1")
            b1 = pers.tile([P, H], f32, tag="b1", name="b1")
            wc = [pers.tile([H, H if l < 2 else C], bf16, tag=f"wc{l}",
                            name=f"wc{l}") for l in range(3)]
            bng = [pers.tile([1, H if l < 2 else C], f32, tag=f"bng{l}",
                             name=f"bng{l}") for l in range(3)]
            bnb = [pers.tile([1, H if l < 2 else C], f32, tag=f"bnb{l}",
                             name=f"bnb{l}") for l in range(3)]
            w3 = pers.tile([C, C], bf16, tag="w3", name="w3")
            b2 = pers.tile([P, C], f32, tag="b2", name="b2")
            idbf = pers.tile([P, P], bf16, tag="idbf", name="idbf")
            onec = pers.tile([P, 1], f32, tag="onec", name="onec")
            oner = pers.tile([1, P], f32, tag="oner", name="oner")
            hA = pers.tile([P, nb * H], bf16, tag="hA", name="hA")
            hB = pers.tile([P, nb * H], bf16, tag="hB", name="hB")
            h3 = pers.tile([P, nb * C], bf16, tag="h3", name="h3")
            sh64 = pers.tile([P, (nb + 1) * H], bf16, tag="sh64", name="sh64")
            sh16 = pers.tile([P, (nb + 1) * C], bf16, tag="sh16", name="sh16")
            agg = pers.tile([P, nb * H], f32, tag="agg", name="agg")
            delta = pers.tile([P, H], f32, tag="delta", name="delta")
            sT = pers.tile([H, 1], f32, tag="sT", name="sT")
            stat = pers.tile([1, 2 * H], f32, tag="stat", name="stat")
            Lg = pers.tile([P, nb * C], f32, tag="Lg", name="Lg")
            exb = pers.tile([P, nb * C], f32, tag="exb", name="exb")
            mx = pers.tile([P, nb], f32, tag="mx", name="mx")
            se = pers.tile([P, nb], f32, tag="se", name="se")

            shd64 = dram.tile([shard, H], bf16, tag="shd64", name="shd64")
            shd16 = dram.tile([shard, C], bf16, tag="shd16", name="shd16")

            sync = nc.sync
            vec = nc.vector
            act = nc.scalar
            gp = nc.gpsimd
            pe = nc.tensor

            sync.dma_start(out=xt[:], in_=xt_d)
            sync.dma_start(out=dinv[:], in_=dinv_d)
            sync.dma_start(out=w1[:], in_=w1_d)
            sync.dma_start(out=b1[:], in_=b1_d)
            for l in range(3):
                sync.dma_start(out=wc[l][:], in_=wc_d[l])
                sync.dma_start(out=bng[l][:], in_=bng_d[l])
                sync.dma_start(out=bnb[l][:], in_=bnb_d[l])
            sync.dma_start(out=w3[:], in_=w3_d)
            sync.dma_start(out=b2[:], in_=b2_d)
            sync.dma_start(out=idbf[:], in_=idbf_d)
            sync.dma_start(out=onec[:], in_=onec_d)
            sync.dma_start(out=oner[:], in_=oner_d)
            vec.memset(sh64[:, nb * H:], 0.0)
            vec.memset(sh16[:, nb * C:], 0.0)

            # fc1
            for b in range(nb):
                pmm = ps2.tile([P, H], f32, tag="pmm", name="pmm")
                pe.matmul(out=pmm[:], lhsT=xt[:, b * P:(b + 1) * P],
                          rhs=w1[:], start=True, stop=True)
                hsl = hA[:, b * H:(b + 1) * H]
                vec.tensor_add(out=hsl, in0=pmm[:], in1=b1[:])
                vec.tensor_scalar_max(out=hsl, in0=hsl, scalar1=0.0)

            hcur = hA
            qn = 0
            for l in range(3):
                F = H if l < 2 else C
                pl = plans[F]
                sh_sb = sh64 if l < 2 else sh16
                sh_d = shd64 if l < 2 else shd16
                rd, stripe = pl["row_div"], pl["stripe"]

                if l == 0:
                    w_eff = wc[0]
                else:
                    w_eff = work.tile([H, F], bf16, tag="weff", name="weff")
                    vec.tensor_scalar_mul(out=w_eff[:], in0=wc[l][:],
                                          scalar1=sT[:H, 0:1])

                for b in range(nb):
                    pt = ps2.tile([H, P], bf16, tag="pt", name="pt")
                    pe.transpose(out=pt[:], in_=hcur[:, b * H:(b + 1) * H],
                                 identity=idbf[:])
                    ht = work.tile([H, P], bf16, tag="ht", name="ht")
                    vec.tensor_copy(out=ht[:], in_=pt[:])
                    pmm = ps2.tile([P, F], f32, tag="pmm", name="pmm")
                    pe.matmul(out=pmm[:], lhsT=ht[:], rhs=w_eff[:],
                              start=True, stop=True)
                    vec.tensor_scalar_mul(out=sh_sb[:, b * F:(b + 1) * F],
                                          in0=pmm[:], scalar1=dinv[:, b:b + 1])

                sync.dma_start(
                    out=sh_d[:].rearrange("(p b) f -> p b f", p=P),
                    in_=sh_sb[:].rearrange("p (b f) -> p b f", f=F))
                tab = dram.tile([NSH, F], bf16, addr_space="Shared",
                                tag=f"tab{F}", name=f"tab{F}_{l}")
                gp.collective_compute(
                    "AllGather", mybir.AluOpType.bypass, replica_groups=RG,
                    ins=[sh_d[:]], outs=[tab[:]])
                tabv = tab[:].rearrange("(r two) f -> r (two f)", two=rd)

                vec.memset(agg[:, :nb * F], 0.0)
                for (k, b0, nblk, wg, col0) in pl["groups"]:
                    s, pi = divmod(k, rd)
                    cols = nblk * wg
                    src_ap = tabv[s * stripe:(s + 1) * stripe,
                                  pi * F:(pi + 1) * F]
                    gi = gwork.tile([P, 8 * KMAX], i16, tag="gi", name="gi")
                    sync.dma_start(out=gi[:, :8 * cols],
                                   in_=idx_d[F][:, 8 * col0:8 * (col0 + cols)])
                    gt = gwork.tile([P, KMAX * H], bf16, tag="gt", name="gt")
                    c0 = 0
                    while c0 < cols:
                        cc = min(CALL_COLS, cols - c0)
                        _dma_gather_raw(
                            gp,
                            gt[:, c0 * F:(c0 + cc) * F].rearrange(
                                "p (k f) -> p k f", f=F),
                            src_ap, gi[:, 8 * c0:8 * (c0 + cc)],
                            128 * cc, F, 128, qn)
                        qn = (qn + 1) % 4
                        c0 += cc
                    rtmp = work.tile([P, KMAX * H // 4], f32, tag="rtmp",
                                     name="rtmp")
                    vec.reduce_sum(
                        out=rtmp[:, :nblk * F],
                        in_=gt[:, :cols * F].rearrange(
                            "p (n w f) -> p n f w", w=wg, f=F),
                        axis=mybir.AxisListType.X)
                    asl = agg[:, b0 * F:(b0 + nblk) * F]
                    vec.tensor_add(out=asl, in0=asl, in1=rtmp[:, :nblk * F])

                a3 = agg[:, :nb * F].rearrange("p (b f) -> p b f", f=F)
                vec.tensor_tensor(
                    out=a3, in0=a3,
                    in1=dinv[:].unsqueeze(2).to_broadcast([P, nb, F]),
                    op=mybir.AluOpType.mult)

                n_chunks, cw = _stat_chunks(nb, F)
                pst1 = ps1.tile([1, cw], f32, tag="pst1", name="pst1")
                pst2 = ps1.tile([1, cw], f32, tag="pst2", name="pst2")
                for ci in range(n_chunks):
                    asl = agg[:, ci * cw:(ci + 1) * cw]
                    sq = work.tile([P, cw], f32, tag="sq", name="sq")
                    vec.tensor_mul(out=sq[:], in0=asl, in1=asl)
                    pe.matmul(out=pst1[:], lhsT=onec[:], rhs=asl,
                              start=(ci == 0), stop=(ci == n_chunks - 1))
                    pe.matmul(out=pst2[:], lhsT=onec[:], rhs=sq[:],
                              start=(ci == 0), stop=(ci == n_chunks - 1))
                vec.reduce_sum(out=stat[0:1, :F],
                               in_=pst1[:].rearrange("o (k f) -> o f k", f=F),
                               axis=mybir.AxisListType.X)
                vec.reduce_sum(out=stat[0:1, F:2 * F],
                               in_=pst2[:].rearrange("o (k f) -> o f k", f=F),
                               axis=mybir.AxisListType.X)

                std = dram.tile([1, 2 * F], f32, tag="std", name="std")
                stdr = dram.tile([1, 2 * F], f32, addr_space="Shared",
                                 tag="stdr", name="stdr")
                sync.dma_start(out=std[:], in_=stat[:1, :2 * F])
                gp.collective_compute(
                    "AllReduce", mybir.AluOpType.add, replica_groups=RG,
                    ins=[std[:]], outs=[stdr[:]])
                sync.dma_start(out=stat[:1, :2 * F], in_=stdr[:])

                mu = work.tile([1, H], f32, tag="mu", name="mu")
                ex2 = work.tile([1, H], f32, tag="ex2", name="ex2")
                var = work.tile([1, H], f32, tag="var", name="var")
                sv = work.tile([1, H], f32, tag="sv", name="sv")
                dl = work.tile([1, H], f32, tag="dl", name="dl")
                vec.tensor_scalar_mul(out=mu[:1, :F], in0=stat[0:1, :F],
                                      scalar1=1.0 / N)
                vec.tensor_scalar_mul(out=ex2[:1, :F], in0=stat[0:1, F:2 * F],
                                      scalar1=1.0 / N)
                vec.tensor_mul(out=var[:1, :F], in0=mu[:1, :F], in1=mu[:1, :F])
                vec.tensor_sub(out=var[:1, :F], in0=ex2[:1, :F],
                               in1=var[:1, :F])
                vec.tensor_scalar_add(out=var[:1, :F], in0=var[:1, :F],
                                      scalar1=BN_EPS)
                act.sqrt(out=var[:1, :F], in_=var[:1, :F])
                vec.reciprocal(out=sv[:1, :F], in_=var[:1, :F])
                vec.tensor_mul(out=sv[:1, :F], in0=sv[:1, :F],
                               in1=bng[l][:1, :F])
                vec.reciprocal(out=dl[:1, :F], in_=sv[:1, :F])
                vec.tensor_mul(out=dl[:1, :F], in0=dl[:1, :F],
                               in1=bnb[l][:1, :F])
                vec.tensor_sub(out=dl[:1, :F], in0=dl[:1, :F],
                               in1=mu[:1, :F])

                pss = ps1.tile([H, 1], f32, tag="psmall", name="pss")
                pe.matmul(out=pss[:F, :], lhsT=sv[:1, :F],
                          rhs=oner[0:1, 0:1], is_transpose=True,
                          start=True, stop=True)
                vec.tensor_copy(out=sT[:F, :], in_=pss[:F, :])
                psd = ps1.tile([P, H], f32, tag="psmall", name="psd")
                pe.matmul(out=psd[:, :F], lhsT=oner[:1, :], rhs=dl[:1, :F],
                          start=True, stop=True)
                vec.tensor_copy(out=delta[:, :F], in_=psd[:, :F])

                vec.tensor_tensor(
                    out=a3, in0=a3,
                    in1=delta[:, :F].unsqueeze(1).to_broadcast([P, nb, F]),
                    op=mybir.AluOpType.add)
                hnext = (hB if l == 0 else hA) if l < 2 else h3
                vec.tensor_scalar_max(out=hnext[:, :nb * F],
                                      in0=agg[:, :nb * F], scalar1=0.0)
                hcur = hnext

            # tail: fc2 + log_softmax
            w3e = work.tile([C, C], bf16, tag="w3e", name="w3e")
            vec.tensor_scalar_mul(out=w3e[:], in0=w3[:], scalar1=sT[:C, 0:1])
            for b in range(nb):
                pt = ps2.tile([H, P], bf16, tag="pt", name="pt")
                pe.transpose(out=pt[:C, :], in_=h3[:, b * C:(b + 1) * C],
                             identity=idbf[:])
                h3t = work.tile([C, P], bf16, tag="h3t", name="h3t")
                vec.tensor_copy(out=h3t[:], in_=pt[:C, :])
                pmm = ps2.tile([P, H], f32, tag="pmm", name="pmm")
                pe.matmul(out=pmm[:, :C], lhsT=h3t[:], rhs=w3e[:],
                          start=True, stop=True)
                vec.tensor_add(out=Lg[:, b * C:(b + 1) * C], in0=pmm[:, :C],
                               in1=b2[:])
            L3 = Lg[:].rearrange("p (b f) -> p b f", f=C)
            vec.reduce_max(out=mx[:], in_=L3, axis=mybir.AxisListType.X)
            vec.tensor_tensor(out=L3, in0=L3,
                              in1=mx[:].unsqueeze(2).to_broadcast([P, nb, C]),
                              op=mybir.AluOpType.subtract)
            act.activation(out=exb[:], in_=Lg[:],
                           func=mybir.ActivationFunctionType.Exp)
            vec.reduce_sum(out=se[:],
                           in_=exb[:].rearrange("p (b f) -> p b f", f=C),
                           axis=mybir.AxisListType.X)
            act.activation(out=se[:], in_=se[:],
                           func=mybir.ActivationFunctionType.Ln)
            vec.tensor_tensor(out=L3, in0=L3,
                              in1=se[:].unsqueeze(2).to_broadcast([P, nb, C]),
                              op=mybir.AluOpType.subtract)
            sync.dma_start(out=out_d, in_=Lg[:])

    nc.compile()
    return nc


# ------------------------------------------------------------------- drivers

def run_hw(inputs):
    meta, perms, idx_arrays, dinv_sb = prep(np.asarray(inputs["x"]),
                                            np.asarray(inputs["edge_index"]))
    nc = build_program(meta)
    in_maps = make_core_inputs(meta, perms, idx_arrays, dinv_sb, inputs)
    res = run_bass_kernel_spmd(nc, in_maps, list(range(NCORES)))
    return unpack_outputs(meta, perms, res.results), res


# ------------------------------------------------------------------ interface

_CACHE = {}


def kernel(x, edge_index, fc1_w, fc1_b,
           conv0_w, conv0_b, bn0_g, bn0_b,
           conv1_w, conv1_b, bn1_g, bn1_b,
           conv2_w, conv2_b, bn2_g, bn2_b,
           fc2_w, fc2_b):
    """GCN forward on 8 trn2 NeuronCores; takes full inputs, returns full
    [N, 16] log-probs."""
    inputs = dict(x=x, edge_index=edge_index, fc1_w=fc1_w, fc1_b=fc1_b,
                  conv0_w=conv0_w, conv0_b=conv0_b, bn0_g=bn0_g, bn0_b=bn0_b,
                  conv1_w=conv1_w, conv1_b=conv1_b, bn1_g=bn1_g, bn1_b=bn1_b,
                  conv2_w=conv2_w, conv2_b=conv2_b, bn2_g=bn2_g, bn2_b=bn2_b,
                  fc2_w=fc2_w, fc2_b=fc2_b)
    ei = np.asarray(edge_index)
    key = (ei.shape[1], int(ei[0, 0]), int(ei[1, -1]), np.asarray(x).shape[0])
    if key not in _CACHE:
        meta, perms, idxa, dinv_sb = prep(np.asarray(x), ei)
        nc = build_program(meta)
        _CACHE[key] = (meta, perms, idxa, dinv_sb, nc)
    meta, perms, idxa, dinv_sb, nc = _CACHE[key]
    in_maps = make_core_inputs(meta, perms, idxa, dinv_sb, inputs)
    res = run_bass_kernel_spmd(nc, in_maps, list(range(NCORES)))
    out = unpack_outputs(meta, perms, res.results)
    kernel._last_results = res
    return out.astype(np.float32)


# revision 7
# speedup vs baseline: 3.3431x; 3.3431x over previous
"""GCN on 8 trn2 NeuronCores via Bass/Tile.

Layout/algorithm (validated in proto.py):
- Nodes sharded by id range across 8 cores; within a core, nodes are
  degree-sorted (perm pi) and padded to nb*128.
- Table trick: each conv layer builds tableS[v] = dinv[v] * (h[v] @ W) as a
  per-core shard, AllGathers the full bf16 table to DRAM, then each core
  gathers rows for its (dst-sorted, degree-bucketed) edge slots with
  dma_gather and segment-reduces on the vector engine.
- agg[d] = dinv[d] * (sum_slots table[src]); conv bias is a no-op under
  training-mode BN; BN scale folds into the next weight matrix, BN shift
  folds into a per-feature delta added before ReLU.
- BN stats (sum, sum of squares) via ones-matmul on PE + tiny AllReduce.
- Table DRAM layout: per core `shard = (nb+1)*128` rows; node at sorted pos
  i lives at flat row c*shard + (i%128)*(nb+1) + (i//128); rows with b==nb
  are zero rows (padding targets). dma_gather needs a 256-byte HBM row
  stride and int16 (<32768) indices, so flat rows are grouped into packed
  256B rows of row_div = 128//F nodes; indices address packed rows within a
  <=25344-row stripe, and the sub-row is selected by the gather source AP's
  element offset.  Slots are therefore split into
  n_classes = n_stripes*row_div classes; within each (class, group of
  consecutive dst blocks) the per-dst slot count is padded to a uniform
  width so one strided reduce handles the whole group.
- dma_gather quirks (found empirically): idx position i lives at
  [i%16, i//16] of a [128, n/16] int16 SBUF tile and must be replicated
  across all eight 16-partition stripes; <=1024 indices per call; payload
  may be a 64/16-element slice of the 128-element row (elem_step=128).
"""
import numpy as np
import ml_dtypes

import concourse.bass as bass
import concourse.bacc as bacc
import concourse.mybir as mybir
import concourse.tile as tile
from concourse.bass_utils import run_bass_kernel_spmd

BF16 = ml_dtypes.bfloat16
P = 128
NCORES = 8
BN_EPS = 1e-5
KMAX = 96            # max gather-tile columns per (class, group)
CALL_COLS = 8        # columns (=128 idxs each) per dma_gather call
IDX_CAP = 32768      # int16 index space per stripe


# ----------------------------------------------------------------- host prep

def _plan_classes(F, nb, shard, split=False):
    row_div = 128 // F
    packed_rows = NCORES * shard // row_div
    n_stripes = -(-packed_rows // IDX_CAP)
    if split:
        n_stripes = max(2, n_stripes)     # one stripe per AllGather half
        assert n_stripes == 2, "split plan needs exactly 2 stripes"
    assert packed_rows % n_stripes == 0
    stripe = packed_rows // n_stripes
    return dict(F=F, row_div=row_div, n_stripes=n_stripes, stripe=stripe,
                n_classes=n_stripes * row_div)


def _pad_lidx(pl, nb, shard, split, flat_rows):
    """Per class: list of local packed-row idxs that are zero half-rows."""
    row_div, stripe = pl["row_div"], pl["stripe"]
    pads = []
    zc = np.zeros(1, np.int64)
    for k in range(pl["n_classes"]):
        s, pi = divmod(k, row_div)
        lids = []
        for c in range(NCORES):
            for p in range(P):
                r = int(flat_rows(np.full(1, c), np.full(1, p * (nb + 1) + nb),
                                  split)[0])
                if r % row_div == pi and r // row_div // stripe == s:
                    lids.append(r // row_div - s * stripe)
            if lids:
                break
        assert lids, (k, pi)
        pads.append(np.array(lids, np.int64))
    return pads


def prep(x, edge_index):
    N = x.shape[0]
    assert N % NCORES == 0
    n_own = N // NCORES
    nb = (n_own + P - 1) // P
    npad = nb * P
    shard = (nb + 1) * P

    src = np.asarray(edge_index[0], np.int64)
    dst = np.asarray(edge_index[1], np.int64)
    deg = np.bincount(dst, minlength=N).astype(np.int64) + 1
    dinv = (1.0 / np.sqrt(deg.astype(np.float64))).astype(np.float32)

    perms = []
    inv_pos = np.empty(N, np.int64)
    for c in range(NCORES):
        lo = c * n_own
        order = np.argsort(-deg[lo:lo + n_own], kind="stable")
        perms.append(order)
        inv = np.empty(n_own, np.int64)
        inv[order] = np.arange(n_own)
        inv_pos[lo:lo + n_own] = inv

    # local shard row of each global node
    gi = np.arange(N)
    rloc = ((inv_pos % P) * (nb + 1) + inv_pos // P).astype(np.int64)
    gcore = (gi // n_own).astype(np.int64)
    half = shard // 2
    NSH = NCORES * shard

    def flat_rows(cores, rl, split):
        if not split:
            return cores * shard + rl
        return np.where(rl < half, cores * half + rl,
                        NSH // 2 + cores * half + (rl - half))

    # edge arrays sorted by (dst core, dst pos)
    ecore = dst // n_own
    eorder = np.argsort(inv_pos[dst] + (ecore << 32), kind="stable")
    ds = dst[eorder]
    ss = src[eorder]
    e_ci = ds // n_own                      # dst core
    e_i = inv_pos[ds]                       # dst sorted pos

    plans = {}
    idx_arrays = {}
    for F in (64, 16):
        split = (F == 64)
        pl = _plan_classes(F, nb, shard, split)
        pl["split"] = split
        rd, stripe, ncl = pl["row_div"], pl["stripe"], pl["n_classes"]
        pads = _pad_lidx(pl, nb, shard, split, flat_rows)

        e_r = flat_rows(gcore[ss], rloc[ss], split)
        e_pk = e_r // rd
        e_s = e_pk // stripe
        e_k = e_s * rd + e_r % rd           # class of each edge's src
        e_l = e_pk - e_s * stripe           # local idx within stripe

        self_r = np.empty((NCORES, npad), np.int64)
        i = np.arange(npad)
        rl_i = (i % P) * (nb + 1) + i // P
        for c in range(NCORES):
            self_r[c] = flat_rows(np.full(npad, c), rl_i, split)
        self_pk = self_r // rd
        self_s = self_pk // stripe
        self_k = self_s * rd + self_r % rd
        self_l = self_pk - self_s * stripe

        # per-(core, node, class) counts
        cnt = np.zeros((NCORES, npad, ncl), np.int32)
        np.add.at(cnt, (e_ci, e_i, e_k), 1)
        real = np.arange(npad) < n_own
        cnt[:, real, :][...] = cnt[:, real, :]          # no-op, clarity
        for c in range(NCORES):
            cnt[c, real, self_k[c, real]] += 1

        # uniform widths per (class, block) over all cores
        Wbk = cnt.reshape(NCORES, nb, P, ncl).max(axis=(0, 2))   # [nb, ncl]

        # group packing per class: uniform width per group, <= KMAX cols
        groups = []          # (k, b0, nblk, Wg, col0)
        col0 = 0
        for k in range(ncl):
            b = 0
            while b < nb:
                b0 = b
                wmax = max(int(Wbk[b, k]), 1)
                n = 1
                b += 1
                while b < nb and n < 24:
                    w2 = max(wmax, int(Wbk[b, k]))
                    if (n + 1) * w2 > KMAX:
                        break
                    wmax = w2
                    n += 1
                    b += 1
                groups.append((k, b0, n, wmax, col0))
                col0 += n * wmax
        T = col0

        # column base for each (core-independent) (class, block)
        colbase = np.full((ncl, nb), -1, np.int64)
        gwidth = np.zeros((ncl, nb), np.int64)
        for (k, b0, n, wg, c0) in groups:
            for j in range(n):
                colbase[k, b0 + j] = c0 + j * wg
                gwidth[k, b0 + j] = wg

        # fill idx columns per core
        idxs = np.empty((NCORES, P, T), np.int16)
        padcols = np.empty(T, np.int16)
        for (kk, b0, n, wg, c0) in groups:
            padv = pads[kk]
            w = np.arange(n * wg)
            padcols[c0:c0 + n * wg] = padv[w % len(padv)].astype(np.int16)
        for c in range(NCORES):
            idxs[c][:] = padcols[None, :]
            # self slots
            i = np.arange(n_own)
            b = i // P
            p = i % P
            k = self_k[c, :n_own]
            idxs[c][p, colbase[k, b]] = self_l[c, :n_own].astype(np.int16)
        # edge slots (vectorized across all cores)
        key2 = (e_ci << 56) + (e_i << 8) + e_k
        o2 = np.argsort(key2, kind="stable")
        k2 = e_k[o2]
        ci2 = e_ci[o2]
        i2 = e_i[o2]
        l2 = e_l[o2]
        kk2 = key2[o2]
        newrun = np.concatenate([[True], kk2[1:] != kk2[:-1]])
        starts = np.flatnonzero(newrun)
        run_id = np.cumsum(newrun) - 1
        rank = np.arange(len(o2)) - starts[run_id]
        is_self_cls = self_k[ci2, i2] == k2
        col = colbase[k2, i2 // P] + rank + is_self_cls
        idxs[ci2, i2 % P, col] = l2.astype(np.int16)

        # pack for dma_gather: position p of column j -> [p%16, 8j + p//16],
        # replicated across the 8 partition stripes
        packed = np.empty((NCORES, 16, 8 * T), np.int16)
        for c in range(NCORES):
            a = idxs[c].reshape(P, T)                       # [128, T]
            b = a.reshape(8, 16, T).transpose(1, 0, 2)      # [16, 8, T] (a=p%16,t=p//16)
            packed[c] = b.transpose(0, 2, 1).reshape(16, T * 8)

        pl["groups"] = groups
        pl["T"] = T
        plans[F] = pl
        idx_arrays[F] = packed

    dinv_sb = np.zeros((NCORES, P, nb), np.float32)
    for c in range(NCORES):
        d = dinv[c * n_own:(c + 1) * n_own][perms[c]]
        dpad = np.concatenate([d, np.zeros(npad - n_own, np.float32)])
        dinv_sb[c] = dpad.reshape(nb, P).T

    meta = dict(N=N, n_own=n_own, nb=nb, npad=npad, shard=shard, plans=plans)
    return meta, perms, idx_arrays, dinv_sb


def make_core_inputs(meta, perms, idx_arrays, dinv_sb, inputs):
    x = np.asarray(inputs["x"], np.float32)
    n_own, nb, npad = meta["n_own"], meta["nb"], meta["npad"]
    DIN = x.shape[1]

    shared = {
        "w1": np.asarray(inputs["fc1_w"], np.float32).astype(BF16),
        "b1": np.tile(np.asarray(inputs["fc1_b"], np.float32), (P, 1)),
        "w3": np.asarray(inputs["fc2_w"], np.float32).astype(BF16),
        "b2": np.tile(np.asarray(inputs["fc2_b"], np.float32), (P, 1)),
        "idbf": np.eye(P, dtype=np.float32).astype(BF16),
        "ones_col": np.ones((P, 1), np.float32),
        "ones_row": np.ones((1, P), np.float32),
    }
    for l in range(3):
        shared[f"wc{l}"] = np.asarray(inputs[f"conv{l}_w"], np.float32).astype(BF16)
        shared[f"bng{l}"] = np.asarray(inputs[f"bn{l}_g"], np.float32)[None, :]
        shared[f"bnb{l}"] = np.asarray(inputs[f"bn{l}_b"], np.float32)[None, :]

    in_maps = []
    for c in range(NCORES):
        xs = x[c * n_own:(c + 1) * n_own][perms[c]]
        xs = np.concatenate([xs, np.zeros((npad - n_own, DIN), np.float32)])
        m = dict(shared)
        m["x_t"] = np.ascontiguousarray(xs.T).astype(BF16)
        m["idx64"] = np.ascontiguousarray(idx_arrays[64][c])
        m["idx16"] = np.ascontiguousarray(idx_arrays[16][c])
        m["dinv"] = np.ascontiguousarray(dinv_sb[c])
        in_maps.append(m)
    return in_maps


def unpack_outputs(meta, perms, results):
    N, n_own, nb = meta["N"], meta["n_own"], meta["nb"]
    C = 16
    out = np.empty((N, C), np.float32)
    for c in range(NCORES):
        o = np.asarray(results[c]["out"])
        o = o.reshape(P, nb, C).transpose(1, 0, 2).reshape(nb * P, C)[:n_own]
        tmp = np.empty((n_own, C), np.float32)
        tmp[perms[c]] = o
        out[c * n_own:(c + 1) * n_own] = tmp
    return out


# ------------------------------------------------------------ device program

def _stat_chunks(nb, F):
    kmax = 512 // F
    k = max(d for d in range(1, kmax + 1) if nb % d == 0)
    return nb // k, k * F


def _dma_gather_raw(gp, out_ap, in_ap, idxs_ap, num_idxs, elem_size,
                    elem_step, queue_num):
    """bass.dma_gather minus the elem_size%256 assert: payload may be a
    slice of the 256B-strided row."""
    stride_bytes = elem_step * mybir.dt.size(in_ap.dtype)
    assert stride_bytes % 256 == 0 and stride_bytes // 256 < 256
    assert idxs_ap.dtype == mybir.dt.int16
    return gp.add_instruction(
        mybir.InstDMAGatherAnt(
            name=gp.bass.get_next_instruction_name(),
            ins=[*gp.lower_ap_dma(in_ap, for_custom_bir_dma=True),
                 gp.lower_ap(idxs_ap),
                 gp.lower_val_access(gp.to_reg(num_idxs))],
            outs=[gp.lower_ap(out_ap)],
            transpose=False, num_idxs=num_idxs, elem_size=elem_size,
            stride_bytes_256=stride_bytes // 256, gen_mode=0,
            single_packet=True, queue_num=queue_num,
            sbuf_tokens_per_rank=0, sbuf_free_dim_per_rank=0,
            sbuf_free_dim_pad_per_rank=0, sbuf_byte_offset=0))


def build_program(meta, DIN=128, H=64, C=16):
    nb, npad, shard = meta["nb"], meta["npad"], meta["shard"]
    N = meta["N"]
    plans = meta["plans"]
    NSH = NCORES * shard
    dt = mybir.dt
    f32, bf16, i16 = dt.float32, dt.bfloat16, dt.int16
    RG = [list(range(NCORES))]

    nc = bacc.Bacc("TRN2", target_bir_lowering=False, debug=False,
                   num_devices=NCORES, num_swdge_queues=4)

    def din(name, shape, dtype):
        return nc.dram_tensor(name, shape, dtype, kind="ExternalInput").ap()

    xt_d = din("x_t", [DIN, npad], bf16)
    idx_d = {F: din(f"idx{F}", [16, 8 * plans[F]["T"]], i16) for F in (64, 16)}
    dinv_d = din("dinv", [P, nb], f32)
    w1_d = din("w1", [DIN, H], bf16)
    b1_d = din("b1", [P, H], f32)
    wc_d = [din(f"wc{l}", [H, H if l < 2 else C], bf16) for l in range(3)]
    bng_d = [din(f"bng{l}", [1, H if l < 2 else C], f32) for l in range(3)]
    bnb_d = [din(f"bnb{l}", [1, H if l < 2 else C], f32) for l in range(3)]
    w3_d = din("w3", [C, C], bf16)
    b2_d = din("b2", [P, C], f32)
    idbf_d = din("idbf", [P, P], bf16)
    onec_d = din("ones_col", [P, 1], f32)
    oner_d = din("ones_row", [1, P], f32)
    out_d = nc.dram_tensor("out", [P, nb * C], f32, kind="ExternalOutput").ap()

    with tile.TileContext(nc) as tc:
        with tc.tile_pool(name="pers", bufs=1) as pers, \
             tc.tile_pool(name="work", bufs=2) as work, \
             tc.tile_pool(name="gwork", bufs=3) as gwork, \
             tc.tile_pool(name="ps2", bufs=2, space="PSUM") as ps2, \
             tc.tile_pool(name="ps1", bufs=1, space="PSUM") as ps1, \
             tc.tile_pool(name="dram", bufs=1, space="DRAM") as dram:

            xt = pers.tile([DIN, npad], bf16, tag="xt", name="xt")
            dinv = pers.tile([P, nb], f32, tag="dinv", name="dinv")
            w1 = pers.tile([DIN, H], bf16, tag="w1", name="w1")
            b1 = pers.tile([P, H], f32, tag="b1", name="b1")
            wc = [pers.tile([H, H if l < 2 else C], bf16, tag=f"wc{l}",
                            name=f"wc{l}") for l in range(3)]
            bng = [pers.tile([1, H if l < 2 else C], f32, tag=f"bng{l}",
                             name=f"bng{l}") for l in range(3)]
            bnb = [pers.tile([1, H if l < 2 else C], f32, tag=f"bnb{l}",
                             name=f"bnb{l}") for l in range(3)]
            w3 = pers.tile([C, C], bf16, tag="w3", name="w3")
            b2 = pers.tile([P, C], f32, tag="b2", name="b2")
            idbf = pers.tile([P, P], bf16, tag="idbf", name="idbf")
            onec = pers.tile([P, 1], f32, tag="onec", name="onec")
            oner = pers.tile([1, P], f32, tag="oner", name="oner")
            hA = pers.tile([P, nb * H], bf16, tag="hA", name="hA")
            hB = pers.tile([P, nb * H], bf16, tag="hB", name="hB")
            h3 = pers.tile([P, nb * C], bf16, tag="h3", name="h3")
            sh64 = pers.tile([P, (nb + 1) * H], bf16, tag="sh64", name="sh64")
            sh16 = pers.tile([P, (nb + 1) * C], bf16, tag="sh16", name="sh16")
            agg = pers.tile([P, nb * H], f32, tag="agg", name="agg")
            delta = pers.tile([P, H], f32, tag="delta", name="delta")
            sT = pers.tile([H, 1], f32, tag="sT", name="sT")
            stat = pers.tile([1, 2 * H], f32, tag="stat", name="stat")
            Lg = pers.tile([P, nb * C], f32, tag="Lg", name="Lg")
            exb = pers.tile([P, nb * C], f32, tag="exb", name="exb")
            mx = pers.tile([P, nb], f32, tag="mx", name="mx")
            se = pers.tile([P, nb], f32, tag="se", name="se")

            shd64 = dram.tile([shard, H], bf16, tag="shd64", name="shd64")
            shd16 = dram.tile([shard, C], bf16, tag="shd16", name="shd16")
            idxx = {F: dram.tile([P, 8 * plans[F]["T"]], i16, tag=f"idxx{F}",
                                 name=f"idxx{F}") for F in (64, 16)}

            sync = nc.sync
            vec = nc.vector
            act = nc.scalar
            gp = nc.gpsimd
            pe = nc.tensor

            sync.dma_start(out=xt[:], in_=xt_d)
            sync.dma_start(out=dinv[:], in_=dinv_d)
            sync.dma_start(out=w1[:], in_=w1_d)
            sync.dma_start(out=b1[:], in_=b1_d)
            for l in range(3):
                sync.dma_start(out=wc[l][:], in_=wc_d[l])
                sync.dma_start(out=bng[l][:], in_=bng_d[l])
                sync.dma_start(out=bnb[l][:], in_=bnb_d[l])
            sync.dma_start(out=w3[:], in_=w3_d)
            sync.dma_start(out=b2[:], in_=b2_d)
            sync.dma_start(out=idbf[:], in_=idbf_d)
            sync.dma_start(out=onec[:], in_=onec_d)
            sync.dma_start(out=oner[:], in_=oner_d)
            vec.memset(sh64[:, nb * H:], 0.0)
            vec.memset(sh16[:, nb * C:], 0.0)
            # replicate the [16, 8T] idx arrays across the 8 partition
            # stripes once, in DRAM (dma_gather's Q7 cores each read their
            # own 16-partition stripe)
            for F in (64, 16):
                for m in range(8):
                    sync.dma_start(out=idxx[F][16 * m:16 * (m + 1), :],
                                   in_=idx_d[F])

            # fc1
            for b in range(nb):
                pmm = ps2.tile([P, H], f32, tag="pmm", name="pmm")
                pe.matmul(out=pmm[:], lhsT=xt[:, b * P:(b + 1) * P],
                          rhs=w1[:], start=True, stop=True)
                hsl = hA[:, b * H:(b + 1) * H]
                vec.tensor_add(out=hsl, in0=pmm[:], in1=b1[:])
                vec.tensor_scalar_max(out=hsl, in0=hsl, scalar1=0.0)

            hcur = hA
            qn = 0
            for l in range(3):
                F = H if l < 2 else C
                pl = plans[F]
                sh_sb = sh64 if l < 2 else sh16
                sh_d = shd64 if l < 2 else shd16
                rd, stripe = pl["row_div"], pl["stripe"]

                if l == 0:
                    w_eff = wc[0]
                else:
                    w_eff = work.tile([H, F], bf16, tag="weff", name="weff")
                    vec.tensor_scalar_mul(out=w_eff[:], in0=wc[l][:],
                                          scalar1=sT[:H, 0:1])

                for b in range(nb):
                    pt = ps2.tile([H, P], bf16, tag="pt", name="pt")
                    pe.transpose(out=pt[:], in_=hcur[:, b * H:(b + 1) * H],
                                 identity=idbf[:])
                    ht = work.tile([H, P], bf16, tag="ht", name="ht")
                    vec.tensor_copy(out=ht[:], in_=pt[:])
                    pmm = ps2.tile([P, F], f32, tag="pmm", name="pmm")
                    pe.matmul(out=pmm[:], lhsT=ht[:], rhs=w_eff[:],
                              start=True, stop=True)
                    vec.tensor_scalar_mul(out=sh_sb[:, b * F:(b + 1) * F],
                                          in0=pmm[:], scalar1=dinv[:, b:b + 1])

                half = shard // 2
                if pl["split"]:
                    sync.dma_start(
                        out=sh_d[:half].rearrange("(p b) f -> p b f", p=P // 2),
                        in_=sh_sb[:P // 2].rearrange("p (b f) -> p b f", f=F))
                    sync.dma_start(
                        out=sh_d[half:].rearrange("(p b) f -> p b f", p=P // 2),
                        in_=sh_sb[P // 2:].rearrange("p (b f) -> p b f", f=F))
                    tabs = []
                    for hh in range(2):
                        tabh = dram.tile([NSH // 2, F], bf16,
                                         addr_space="Shared",
                                         tag=f"tab{F}h{hh}",
                                         name=f"tab{F}_{l}h{hh}")
                        gp.collective_compute(
                            "AllGather", mybir.AluOpType.bypass,
                            replica_groups=RG,
                            ins=[sh_d[half * hh:half * (hh + 1)]],
                            outs=[tabh[:]])
                        tabs.append(tabh[:].rearrange(
                            "(r two) f -> r (two f)", two=rd))
                else:
                    sync.dma_start(
                        out=sh_d[:].rearrange("(p b) f -> p b f", p=P),
                        in_=sh_sb[:].rearrange("p (b f) -> p b f", f=F))
                    tab = dram.tile([NSH, F], bf16, addr_space="Shared",
                                    tag=f"tab{F}", name=f"tab{F}_{l}")
                    gp.collective_compute(
                        "AllGather", mybir.AluOpType.bypass, replica_groups=RG,
                        ins=[sh_d[:]], outs=[tab[:]])
                    tabv = tab[:].rearrange("(r two) f -> r (two f)", two=rd)

                vec.memset(agg[:, :nb * F], 0.0)
                for (k, b0, nblk, wg, col0) in pl["groups"]:
                    s, pi = divmod(k, rd)
                    cols = nblk * wg
                    if pl["split"]:
                        src_ap = tabs[s][:, pi * F:(pi + 1) * F]
                    else:
                        src_ap = tabv[s * stripe:(s + 1) * stripe,
                                      pi * F:(pi + 1) * F]
                    gi = gwork.tile([P, 8 * KMAX], i16, tag="gi", name="gi")
                    sync.dma_start(out=gi[:, :8 * cols],
                                   in_=idxx[F][:, 8 * col0:8 * (col0 + cols)])
                    gt = gwork.tile([P, KMAX * H], bf16, tag="gt", name="gt")
                    c0 = 0
                    while c0 < cols:
                        cc = min(CALL_COLS, cols - c0)
                        _dma_gather_raw(
                            gp,
                            gt[:, c0 * F:(c0 + cc) * F].rearrange(
                                "p (k f) -> p k f", f=F),
                            src_ap, gi[:, 8 * c0:8 * (c0 + cc)],
                            128 * cc, F, 128, qn)
                        qn = (qn + 1) % 4
                        c0 += cc
                    rtmp = work.tile([P, KMAX * H // 4], f32, tag="rtmp",
                                     name="rtmp")
                    vec.reduce_sum(
                        out=rtmp[:, :nblk * F],
                        in_=gt[:, :cols * F].rearrange(
                            "p (n w f) -> p n f w", w=wg, f=F),
                        axis=mybir.AxisListType.X)
                    asl = agg[:, b0 * F:(b0 + nblk) * F]
                    vec.tensor_add(out=asl, in0=asl, in1=rtmp[:, :nblk * F])

                a3 = agg[:, :nb * F].rearrange("p (b f) -> p b f", f=F)
                vec.tensor_tensor(
                    out=a3, in0=a3,
                    in1=dinv[:].unsqueeze(2).to_broadcast([P, nb, F]),
                    op=mybir.AluOpType.mult)

                n_chunks, cw = _stat_chunks(nb, F)
                pst1 = ps1.tile([1, cw], f32, tag="pst1", name="pst1")
                pst2 = ps1.tile([1, cw], f32, tag="pst2", name="pst2")
                for ci in range(n_chunks):
                    asl = agg[:, ci * cw:(ci + 1) * cw]
                    sq = work.tile([P, cw], f32, tag="sq", name="sq")
                    vec.tensor_mul(out=sq[:], in0=asl, in1=asl)
                    pe.matmul(out=pst1[:], lhsT=onec[:], rhs=asl,
                              start=(ci == 0), stop=(ci == n_chunks - 1))
                    pe.matmul(out=pst2[:], lhsT=onec[:], rhs=sq[:],
                              start=(ci == 0), stop=(ci == n_chunks - 1))
                vec.reduce_sum(out=stat[0:1, :F],
                               in_=pst1[:].rearrange("o (k f) -> o f k", f=F),
                               axis=mybir.AxisListType.X)
                vec.reduce_sum(out=stat[0:1, F:2 * F],
                               in_=pst2[:].rearrange("o (k f) -> o f k", f=F),
                               axis=mybir.AxisListType.X)

                std = dram.tile([1, 2 * F], f32, tag="std", name="std")
                stdr = dram.tile([1, 2 * F], f32, addr_space="Shared",
                                 tag="stdr", name="stdr")
                sync.dma_start(out=std[:], in_=stat[:1, :2 * F])
                gp.collective_compute(
                    "AllReduce", mybir.AluOpType.add, replica_groups=RG,
                    ins=[std[:]], outs=[stdr[:]])
                sync.dma_start(out=stat[:1, :2 * F], in_=stdr[:])

                mu = work.tile([1, H], f32, tag="mu", name="mu")
                ex2 = work.tile([1, H], f32, tag="ex2", name="ex2")
                var = work.tile([1, H], f32, tag="var", name="var")
                sv = work.tile([1, H], f32, tag="sv", name="sv")
                dl = work.tile([1, H], f32, tag="dl", name="dl")
                vec.tensor_scalar_mul(out=mu[:1, :F], in0=stat[0:1, :F],
                                      scalar1=1.0 / N)
                vec.tensor_scalar_mul(out=ex2[:1, :F], in0=stat[0:1, F:2 * F],
                                      scalar1=1.0 / N)
                vec.tensor_mul(out=var[:1, :F], in0=mu[:1, :F], in1=mu[:1, :F])
                vec.tensor_sub(out=var[:1, :F], in0=ex2[:1, :F],
                               in1=var[:1, :F])
                vec.tensor_scalar_add(out=var[:1, :F], in0=var[:1, :F],
                                      scalar1=BN_EPS)
                act.sqrt(out=var[:1, :F], in_=var[:1, :F])
                vec.reciprocal(out=sv[:1, :F], in_=var[:1, :F])
                vec.tensor_mul(out=sv[:1, :F], in0=sv[:1, :F],
                               in1=bng[l][:1, :F])
                vec.reciprocal(out=dl[:1, :F], in_=sv[:1, :F])
                vec.tensor_mul(out=dl[:1, :F], in0=dl[:1, :F],
                               in1=bnb[l][:1, :F])
                vec.tensor_sub(out=dl[:1, :F], in0=dl[:1, :F],
                               in1=mu[:1, :F])

                pss = ps1.tile([H, 1], f32, tag="psmall", name="pss")
                pe.matmul(out=pss[:F, :], lhsT=sv[:1, :F],
                          rhs=oner[0:1, 0:1], is_transpose=True,
                          start=True, stop=True)
                vec.tensor_copy(out=sT[:F, :], in_=pss[:F, :])
                psd = ps1.tile([P, H], f32, tag="psmall", name="psd")
                pe.matmul(out=psd[:, :F], lhsT=oner[:1, :], rhs=dl[:1, :F],
                          start=True, stop=True)
                vec.tensor_copy(out=delta[:, :F], in_=psd[:, :F])

                vec.tensor_tensor(
                    out=a3, in0=a3,
                    in1=delta[:, :F].unsqueeze(1).to_broadcast([P, nb, F]),
                    op=mybir.AluOpType.add)
                hnext = (hB if l == 0 else hA) if l < 2 else h3
                vec.tensor_scalar_max(out=hnext[:, :nb * F],
                                      in0=agg[:, :nb * F], scalar1=0.0)
                hcur = hnext

            # tail: fc2 + log_softmax
            w3e = work.tile([C, C], bf16, tag="w3e", name="w3e")
            vec.tensor_scalar_mul(out=w3e[:], in0=w3[:], scalar1=sT[:C, 0:1])
            for b in range(nb):
                pt = ps2.tile([H, P], bf16, tag="pt", name="pt")
                pe.transpose(out=pt[:C, :], in_=h3[:, b * C:(b + 1) * C],
                             identity=idbf[:])
                h3t = work.tile([C, P], bf16, tag="h3t", name="h3t")
                vec.tensor_copy(out=h3t[:], in_=pt[:C, :])
                pmm = ps2.tile([P, H], f32, tag="pmm", name="pmm")
                pe.matmul(out=pmm[:, :C], lhsT=h3t[:], rhs=w3e[:],
                          start=True, stop=True)
                vec.tensor_add(out=Lg[:, b * C:(b + 1) * C], in0=pmm[:, :C],
                               in1=b2[:])
            L3 = Lg[:].rearrange("p (b f) -> p b f", f=C)
            vec.reduce_max(out=mx[:], in_=L3, axis=mybir.AxisListType.X)
            vec.tensor_tensor(out=L3, in0=L3,
                              in1=mx[:].unsqueeze(2).to_broadcast([P, nb, C]),
                              op=mybir.AluOpType.subtract)
            act.activation(out=exb[:], in_=Lg[:],
                           func=mybir.ActivationFunctionType.Exp)
            vec.reduce_sum(out=se[:],
                           in_=exb[:].rearrange("p (b f) -> p b f", f=C),
                           axis=mybir.AxisListType.X)
            act.activation(out=se[:], in_=se[:],
                           func=mybir.ActivationFunctionType.Ln)
            vec.tensor_tensor(out=L3, in0=L3,
                              in1=se[:].unsqueeze(2).to_broadcast([P, nb, C]),
                              op=mybir.AluOpType.subtract)
            sync.dma_start(out=out_d, in_=Lg[:])

    nc.compile()
    return nc


# ------------------------------------------------------------------- drivers

def run_hw(inputs):
    meta, perms, idx_arrays, dinv_sb = prep(np.asarray(inputs["x"]),
                                            np.asarray(inputs["edge_index"]))
    nc = build_program(meta)
    in_maps = make_core_inputs(meta, perms, idx_arrays, dinv_sb, inputs)
    res = run_bass_kernel_spmd(nc, in_maps, list(range(NCORES)))
    return unpack_outputs(meta, perms, res.results), res


# ------------------------------------------------------------------ interface

_CACHE = {}


def kernel(x, edge_index, fc1_w, fc1_b,
           conv0_w, conv0_b, bn0_g, bn0_b,
           conv1_w, conv1_b, bn1_g, bn1_b,
           conv2_w, conv2_b, bn2_g, bn2_b,
           fc2_w, fc2_b):
    """GCN forward on 8 trn2 NeuronCores; takes full inputs, returns full
    [N, 16] log-probs."""
    inputs = dict(x=x, edge_index=edge_index, fc1_w=fc1_w, fc1_b=fc1_b,
                  conv0_w=conv0_w, conv0_b=conv0_b, bn0_g=bn0_g, bn0_b=bn0_b,
                  conv1_w=conv1_w, conv1_b=conv1_b, bn1_g=bn1_g, bn1_b=bn1_b,
                  conv2_w=conv2_w, conv2_b=conv2_b, bn2_g=bn2_g, bn2_b=bn2_b,
                  fc2_w=fc2_w, fc2_b=fc2_b)
    ei = np.asarray(edge_index)
    key = (ei.shape[1], int(ei[0, 0]), int(ei[1, -1]), np.asarray(x).shape[0])
    if key not in _CACHE:
        meta, perms, idxa, dinv_sb = prep(np.asarray(x), ei)
        nc = build_program(meta)
        _CACHE[key] = (meta, perms, idxa, dinv_sb, nc)
    meta, perms, idxa, dinv_sb, nc = _CACHE[key]
    in_maps = make_core_inputs(meta, perms, idxa, dinv_sb, inputs)
    res = run_bass_kernel_spmd(nc, in_maps, list(range(NCORES)))
    out = unpack_outputs(meta, perms, res.results)
    kernel._last_results = res
    return out.astype(np.float32)
